# revision 21
# baseline (speedup 1.0000x reference)
"""Trainium2 Bass kernel for nn_Conv_MS_MSA (spectral multi-head self-attention).

Reference computation (per batch):
  qkv = dw3x3_depthwise(conv1x1(x))          # 256 -> 768 ch, then per-ch 3x3
  q, k, v = split(qkv); v_out = v
  per head (8 heads x 32 d): L2-normalize q,k rows over the 65536 pixels,
  attn = softmax(k_norm @ q_norm^T * rescale), out = attn @ v
  out_c = conv3x3_dense(out, w_proj)         # 256 -> 256 ch

Sharding: spatial bands. Core i owns image rows [32i, 32i+32) of BOTH batches,
with halo rows for the two 3x3 convs. The only global coupling is the per-head
32x32 Gram matrices and q/k row norms -- tiny sums over pixels -- reduced with
one ~70KB on-device AllReduce mid-kernel. Everything else is band-local.

Key implementation choices (v2):
- channels on SBUF partitions, pixels on the free dim; matmuls fp32r/bf16.
- q/k transposed for the Gram via DMA xbar transposes (frees PE + ACT).
- Gram and q/k sumsq accumulate in PSUM across a whole batch (kq gram plus
  qq/kk self blocks whose diagonal is the sumsq); extracted once per batch.
- The post-softmax attention matrix (block-diagonal per 4-head group) is
  FOLDED into the 3x3 proj weights (w' = A @ w per tap), so the V pass feeds
  the proj conv directly from the bf16 v band: no attn@v matmul, no out band.
- depthwise taps are spread across PE (diagonal matmuls into PSUM), DVE
  (mul+add pairs; the PSUM-fold op), ACT (scale-mul products) and GPSIMD
  (scalar_tensor_tensor) to balance engine load.
"""

import sys

if "/opt/trn_rl_repo" not in sys.path:
    sys.path.insert(0, "/opt/trn_rl_repo")

import numpy as np

import concourse.bass as bass
import concourse.tile as tile
from concourse import bacc, mybir
from concourse import bass_utils

# ---------------------------------------------------------------- problem dims
B = 2
C = 256
H = 256
W = 256
HEADS = 8
N_CORES = 8
ROWS = H // N_CORES          # 32 owned rows per core
VB = ROWS + 2                # 34 v/out band rows (1-row halo each side)
XB = ROWS + 4                # 36 x/qkv band rows (2-row halo each side)
CT = C // 128                # 2 channel tiles of 128 per 256-ch tensor
QKCT = 4                     # q,k channel tiles (512 ch)
EPS = 1e-12

fp32 = mybir.dt.float32
fp32r = mybir.dt.float32r
bf16 = mybir.dt.bfloat16

# ------------------------------------------------- depthwise tap assignment
# tap = dy*3+dx. Per-engine costs per [128, 2048] tile-chunk (cost model):
# PE diag-matmul 853ns; DVE mul(4x)+add(2x) pair 1721ns; ACT scale-mul 1892ns
# + DVE add 1127ns; Pool scale-mul 2939ns + DVE add (neuronxcc rejects
# gpsimd STT, so Pool can only assist with products); DVE STT psum-fold
# (init) 658ns per 2 rows.
QK_PE_TAPS = [0, 2, 5, 7]
QK_INIT = 3                  # DVE STT that also folds in the PE-tap psum
QK_ACT_TAPS = [1, 4]
QK_DVE_PAIRS = [6, 8]
QK_GP_TAPS = []

V_PE_TAPS = [0, 1, 3]
V_INIT = 2
V_ACT_TAPS = []
V_DVE_PAIRS = [5]
V_GP_TAPS = [4, 6, 7, 8]     # Pool products, DVE adds

CHUNK = 8                    # QK-pass rows per chunk
VCHUNK = 8                   # V-pass rows per chunk (non-overlapping over VB)

Alu = mybir.AluOpType
Act = mybir.ActivationFunctionType

_CONST_POOL = None


def _single(tc, shape, dtype, name):
    return _CONST_POOL.tile(shape, dtype, tag=name, name=name)


def _chunks(total, step):
    out = []
    s = 0
    while s < total:
        out.append((s, min(step, total - s)))
        s += step
    return out


def build_program():
    nc = bacc.Bacc(
        "TRN2", target_bir_lowering=False, debug=False, num_devices=N_CORES
    )

    # ------------------------------------------------------------- DRAM I/O
    x_d = nc.dram_tensor("x", [B, CT, 128, XB, 256], fp32r, kind="ExternalInput")
    wq_d = nc.dram_tensor("wq", [128, CT, 768], fp32r, kind="ExternalInput")
    wdw_d = nc.dram_tensor("wdw", [128, 6, 9], fp32, kind="ExternalInput")
    wp_d = nc.dram_tensor("wp", [128, CT, 9, 256], fp32r, kind="ExternalInput")
    ident_d = nc.dram_tensor("ident", [128, 128], fp32r, kind="ExternalInput")
    resc_d = nc.dram_tensor("resc", [128, CT], fp32, kind="ExternalInput")
    hmask_d = nc.dram_tensor("hmask", [128, 2], fp32, kind="ExternalInput")

    vband_d = nc.dram_tensor(
        "vband", [B, CT, 128, VB, 256], bf16, kind="ExternalOutput"
    )
    outc_d = nc.dram_tensor(
        "outc", [B, CT, 128, ROWS, 256], bf16, kind="ExternalOutput"
    )

    with tile.TileContext(nc) as tc:
        global _CONST_POOL
        with tc.tile_pool(name="consts", bufs=1) as cpool:
            _CONST_POOL = cpool
            _build(nc, tc, x_d, wq_d, wdw_d, wp_d, ident_d, resc_d, hmask_d,
                   vband_d, outc_d)
            _CONST_POOL = None
    nc.compile()
    return nc


def _build(nc, tc, x_d, wq_d, wdw_d, wp_d, ident_d, resc_d, hmask_d,
           vband_d, outc_d):
    # ------------------------------------------------------ constants in SBUF
    wq = _single(tc, [128, CT, 768], fp32r, name="wq_sb")
    wdw = _single(tc, [128, 6, 9], fp32, name="wdw_sb")
    wp = _single(tc, [128, CT, 9, 256], fp32r, name="wp_sb")
    ident = _single(tc, [128, 128], fp32r, name="ident_sb")
    resc = _single(tc, [128, CT], fp32, name="resc_sb")
    hmask = _single(tc, [128, 2], fp32, name="hmask_sb")
    nc.sync.dma_start(wq[:], wq_d[:, :, :])
    nc.sync.dma_start(wdw[:], wdw_d[:, :, :])
    nc.sync.dma_start(ident[:], ident_d[:, :])
    nc.sync.dma_start(resc[:], resc_d[:, :])
    nc.sync.dma_start(hmask[:], hmask_d[:, :])

    identb = _single(tc, [128, 128], bf16, name="identb_sb")
    nc.vector.tensor_copy(identb[:], ident[:].bitcast(fp32))

    # diagonal weight matrices for the PE depthwise taps (bf16 everywhere)
    diags = {}
    for t in range(CT):
        for tp in V_PE_TAPS:
            d = _single(tc, [128, 128], bf16, name=f"diag_{t}_{tp}")
            nc.vector.tensor_scalar_mul(
                d[:], identb[:], wdw[:, QKCT + t, tp : tp + 1]
            )
            diags[(QKCT + t, tp)] = d
    for t in range(QKCT):
        for tp in QK_PE_TAPS:
            d = _single(tc, [128, 128], bf16, name=f"diagb_{t}_{tp}")
            nc.vector.tensor_scalar_mul(
                d[:], identb[:], wdw[:, t, tp : tp + 1]
            )
            diags[(t, tp)] = d

    # global stats accumulator (AllReduce payload)
    stats = _single(tc, [128, 136], fp32, name="stats_sb")
    nc.gpsimd.memset(stats[:], 0.0)

    # =========================================================== QK pass
    # owned v-band rows [1, 33): q,k (bf16) via conv1x1 + 9-tap depthwise,
    # then DMA-transpose 128-pixel blocks and accumulate in PSUM:
    #   g_ps[:, g, :]  += kt_g^T @ qt      (kq gram, per 128-ch k block)
    #   s_ps[:, j, :]  += t_j^T @ t_j      (qq/kk self blocks; diag = sumsq)
    with (
        tc.tile_pool(name="xband", bufs=2) as p_x,
        tc.tile_pool(name="qkvt", bufs=5) as p_qkv,
        tc.tile_pool(name="qkp", bufs=6) as p_qk,
        tc.tile_pool(name="qtp", bufs=2) as p_qt,
        tc.tile_pool(name="ascr", bufs=3) as p_ascr,
        tc.tile_pool(name="gscr", bufs=3) as p_gscr,
        tc.tile_pool(name="sqp", bufs=3) as p_sq,
        tc.tile_pool(name="psc", bufs=2, space="PSUM") as ps_conv,
        tc.tile_pool(name="psdq", bufs=2, space="PSUM") as ps_dwq,
        tc.tile_pool(name="pst", bufs=2, space="PSUM") as ps_tr,
        tc.tile_pool(name="psg", bufs=1, space="PSUM") as ps_gram,
    ):
        for b in range(B):
            g_ps = ps_gram.tile([128, 2, 256], fp32, tag="psg", name="g_ps")
            qkchunks = _chunks(ROWS, CHUNK)
            nchunks = len(qkchunks)
            for ci, (s, L) in enumerate(qkchunks):
                s += 1  # band rows [1, 33)
                x_qc = p_x.tile([128, CT, L + 2, 256], fp32r, tag="xq")
                for kt in range(CT):
                    nc.sync.dma_start(
                        x_qc[:, kt], x_d[b, kt][:, s : s + L + 2, :]
                    )
                qk_tiles = []
                for t in range(QKCT):
                    qkv_t = p_qkv.tile([128, L + 2, 258], bf16, tag="qkvt")
                    nc.gpsimd.memset(qkv_t[:, :, 0], 0.0)
                    nc.gpsimd.memset(qkv_t[:, :, 257], 0.0)
                    for n in range((L + 2) // 2):
                        ps = ps_conv.tile([128, 2, 256], fp32, tag="psc")
                        for kt in range(CT):
                            rhs = x_qc[:, kt, 2 * n : 2 * n + 2, :]
                            nc.tensor.matmul(
                                ps[:],
                                wq[:, kt, t * 128 : (t + 1) * 128],
                                rhs,
                                start=(kt == 0),
                                stop=(kt == CT - 1),
                            )
                        nc.scalar.copy(qkv_t[:, 2 * n : 2 * n + 2, 1:257], ps[:])
                    qk_t = p_qk.tile([128, L, 256], bf16, tag="qk")
                    qk3 = qk_t[:, :, :]

                    def win(tp, LL=L, src=qkv_t):
                        dy, dx = tp // 3, tp % 3
                        return src[:, dy : dy + LL, dx : dx + 256]

                    def sc(tp, tt=t):
                        return wdw[:, tt, tp : tp + 1]

                    # PE taps -> psum; DVE init op folds psum + tap QK_INIT
                    dyi, dxi = QK_INIT // 3, QK_INIT % 3
                    for n in range(L // 2):
                        psd = ps_dwq.tile([128, 2, 256], fp32, tag="psdq")
                        for j, tp in enumerate(QK_PE_TAPS):
                            dy, dx = tp // 3, tp % 3
                            nc.tensor.matmul(
                                psd[:],
                                diags[(t, tp)][:],
                                qkv_t[:, 2 * n + dy : 2 * n + dy + 2,
                                      dx : dx + 256],
                                start=(j == 0),
                                stop=(j == len(QK_PE_TAPS) - 1),
                            )
                        nc.vector.scalar_tensor_tensor(
                            qk3[:, 2 * n : 2 * n + 2, :],
                            qkv_t[:, 2 * n + dyi : 2 * n + dyi + 2,
                                  dxi : dxi + 256],
                            sc(QK_INIT),
                            psd[:],
                            op0=Alu.mult, op1=Alu.add,
                        )
                    for tp in QK_ACT_TAPS:
                        ascr = p_ascr.tile([128, L, 256], bf16, tag="ascr")
                        nc.scalar.activation(ascr[:], win(tp), Act.Copy,
                                             scale=sc(tp))
                        nc.vector.tensor_tensor(qk3, qk3, ascr[:], op=Alu.add)
                    for tp in QK_DVE_PAIRS:
                        gscr = p_gscr.tile([128, L, 256], bf16, tag="gscr")
                        nc.vector.tensor_scalar_mul(gscr[:], win(tp), sc(tp))
                        nc.vector.tensor_tensor(qk3, qk3, gscr[:], op=Alu.add)
                    for tp in QK_GP_TAPS:
                        gscr = p_gscr.tile([128, L, 256], bf16, tag="gscr")
                        nc.gpsimd.tensor_scalar_mul(gscr[:], win(tp), sc(tp))
                        nc.vector.tensor_tensor(qk3, qk3, gscr[:], op=Alu.add)
                    # sumsq of this chunk -> stats col 128 + b*4 + t
                    scr = p_gscr.tile([128, L, 256], bf16, tag="gscr")
                    sq = p_sq.tile([128, 1], fp32, tag="sq")
                    nc.scalar.activation(
                        scr[:], qk_t[:], Act.Square, accum_out=sq[:]
                    )
                    col = 128 + b * 4 + t
                    nc.vector.tensor_tensor(
                        stats[:, col : col + 1],
                        stats[:, col : col + 1],
                        sq[:],
                        op=Alu.add,
                    )
                    qk_tiles.append(qk_t)

                # PE transposes (via identity) + Gram accumulated in PSUM
                nblk = (L * 256) // 128
                first = ci == 0
                last = ci == nchunks - 1
                for blk in range(nblk):
                    r, cb = blk // 2, (blk % 2) * 128
                    qt_t = p_qt.tile([128, 256], bf16, tag="qt")
                    kt_t = p_qt.tile([128, 256], bf16, tag="kt")
                    ps_q = ps_tr.tile([128, 256], bf16, tag="pst")
                    ps_k = ps_tr.tile([128, 256], bf16, tag="pst")
                    for half in range(2):
                        nc.tensor.matmul(
                            ps_q[:, half * 128 : half * 128 + 128],
                            qk_tiles[half][:, r, cb : cb + 128],
                            identb[:],
                            is_transpose=True,
                            skip_group_check=True,
                        )
                        nc.tensor.matmul(
                            ps_k[:, half * 128 : half * 128 + 128],
                            qk_tiles[2 + half][:, r, cb : cb + 128],
                            identb[:],
                            is_transpose=True,
                            skip_group_check=True,
                        )
                    nc.scalar.copy(qt_t[:], ps_q[:])
                    nc.scalar.copy(kt_t[:], ps_k[:])
                    st = first and blk == 0
                    sp = last and blk == nblk - 1
                    for g in range(2):
                        nc.tensor.matmul(
                            g_ps[:, g, :],
                            kt_t[:, g * 128 : g * 128 + 128],
                            qt_t[:],
                            start=st, stop=sp,
                            skip_group_check=True,
                        )

            # ---- extract per-head diagonal 32x32 kq blocks -> stats cols
            for g in range(2):
                for i in range(4):
                    h = 4 * g + i
                    nc.vector.tensor_copy(
                        stats[32 * i : 32 * i + 32, (2 * b + g) * 32 :][:, :32],
                        g_ps[32 * i : 32 * i + 32, g, 32 * h : 32 * h + 32],
                    )

    # ============================================================ AllReduce
    with tc.tile_pool(name="ardram", bufs=1, space="DRAM") as p_ar:
        ar_in = p_ar.tile([128, 136], fp32)
        ar_out = p_ar.tile([128, 136], fp32, addr_space="Shared")
        nc.sync.dma_start(ar_in[:], stats[:])
        nc.gpsimd.collective_compute(
            "AllReduce",
            Alu.add,
            replica_groups=[list(range(N_CORES))],
            ins=[ar_in[:].opt()],
            outs=[ar_out[:].opt()],
        )
        # proj weights are first needed at the fold, well after the QK pass:
        # load them while the AllReduce is in flight
        nc.sync.dma_start(wp[:], wp_d[:, :, :, :])
        stats2 = _single(tc, [128, 136], fp32, name="stats2_sb")
        nc.sync.dma_start(stats2[:], ar_out[:])

    # ====================================================== softmax -> attn
    # rsq[:, idx] = 1 / max(sqrt(sumsq), eps), idx = b*4 + qk*2 + g
    rsq = _single(tc, [128, 8], fp32, name="rsq_sb")
    nc.scalar.activation(rsq[:], stats2[:, 128:136], Act.Sqrt)
    nc.vector.tensor_scalar_max(rsq[:], rsq[:], EPS)
    nc.vector.reciprocal(rsq[:], rsq[:])

    bdf = {}
    with tc.tile_pool(name="smx", bufs=4) as p_sm:
        for b in range(B):
            for g in range(2):
                kcol = b * 4 + 2 + g
                qcol = b * 4 + g
                ksc = p_sm.tile([128, 1], fp32, tag="ksc")
                nc.vector.tensor_tensor(
                    ksc[:], rsq[:, kcol : kcol + 1], resc[:, g : g + 1],
                    op=Alu.mult,
                )
                t1 = p_sm.tile([128, 32], fp32, tag="t1")
                graw = stats2[:, (2 * b + g) * 32 :][:, :32]
                nc.vector.tensor_scalar_mul(t1[:], graw, ksc[:])
                # M[p, j] = rsq_q[32*(p//32) + j]: broadcast + block-transpose
                a2 = p_sm.tile([128, 32], fp32, tag="a2")
                nc.vector.tensor_scalar(
                    a2[:], t1[:], 0.0, rsq[:, qcol : qcol + 1],
                    op0=Alu.mult, op1=Alu.add,
                )
                m = p_sm.tile([128, 32], fp32, tag="m")
                nc.vector.transpose(m[:], a2[:])
                nc.vector.tensor_tensor(t1[:], t1[:], m[:], op=Alu.mult)
                # softmax over the free (e) dim
                mx = p_sm.tile([128, 1], fp32, tag="mx")
                nc.vector.tensor_reduce(
                    mx[:], t1[:], mybir.AxisListType.X, Alu.max
                )
                nc.vector.tensor_scalar_sub(t1[:], t1[:], mx[:])
                ex = p_sm.tile([128, 32], fp32, tag="ex")
                nc.scalar.activation(ex[:], t1[:], Act.Exp)
                sm = p_sm.tile([128, 1], fp32, tag="sm")
                nc.vector.tensor_reduce(
                    sm[:], ex[:], mybir.AxisListType.X, Alu.add
                )
                nc.vector.reciprocal(sm[:], sm[:])
                at = p_sm.tile([128, 32], fp32, tag="at")
                nc.vector.tensor_scalar_mul(at[:], ex[:], sm[:])
                # block-diagonal attn (rows=d out-ch, cols=e v-ch) for fold
                bdt = _single(tc, [128, 128], fp32r, name=f"bdf_{b}_{g}")
                nc.gpsimd.memset(bdt[:].bitcast(fp32), 0.0)
                for i in range(4):
                    nc.vector.tensor_copy(
                        bdt[32 * i : 32 * i + 32, 32 * i : 32 * i + 32],
                        at[32 * i : 32 * i + 32, :],
                    )
                bdf[(b, g)] = bdt

    # fold attn into proj weights: w'[j, o, tap] = sum_i attn[i, j] w[i, o, tap]
    wfold = {}
    for b in range(B):
        wf = _single(tc, [128, CT, 9, 256], bf16, name=f"wfold_{b}")
        wfold[b] = wf

    # ================================================= V + proj pass
    with (
        tc.tile_pool(name="xc2", bufs=2) as p_x2,
        tc.tile_pool(name="ut", bufs=4) as p_u,
        tc.tile_pool(name="vbp", bufs=2) as p_vb,
        tc.tile_pool(name="ascr2", bufs=3) as p_ascr2,
        tc.tile_pool(name="gscr2", bufs=3) as p_gscr2,
        tc.tile_pool(name="ocp", bufs=2) as p_oc,
        tc.tile_pool(name="psc2", bufs=2, space="PSUM") as ps_conv2,
        tc.tile_pool(name="psd2", bufs=2, space="PSUM") as ps_dw2,
        tc.tile_pool(name="psp", bufs=4, space="PSUM") as ps_proj,
    ):
        for b in range(B):
            vband = p_vb.tile([128, CT, VB, 258], bf16, tag="vband",
                              name="vband_sb")
            for g in range(CT):
                nc.gpsimd.memset(vband[:, g, :, 0], 0.0)
                nc.gpsimd.memset(vband[:, g, :, 257], 0.0)

            def emit_fold(bb=b):
                # fold the attention into the proj weights (PE + ACT evac)
                for g in range(CT):
                    for tp in range(9):
                        psf = ps_proj.tile([128, 2, 256], fp32, tag="psp",
                                           name="ps_fold")
                        nc.tensor.matmul(
                            psf[:, 0, :],
                            bdf[(bb, g)][:],
                            wp[:, g, tp, :],
                            start=True, stop=True,
                            skip_group_check=True,
                        )
                        nc.scalar.copy(wfold[bb][:, g, tp, :], psf[:, 0, :])

            def proj_group(mt, grp, bb=b, vband=vband):
                # one [2,256] psum bank per output row-pair; weight-major
                # loop so each lhsT loads once for the four banks
                pss = [
                    ps_proj.tile([128, 2, 256], fp32, tag="psp", name="ps_pj")
                    for _ in range(4)
                ]
                idx = 0
                for tp in range(9):
                    dy, dx = tp // 3, tp % 3
                    for kt in range(CT):
                        for j4 in range(4):
                            n = grp * 4 + j4
                            rhs = vband[:, kt,
                                        2 * n + dy : 2 * n + dy + 2,
                                        dx : dx + 256]
                            nc.tensor.matmul(
                                pss[j4][:],
                                wfold[bb][:, kt, tp, mt * 128 :][:, :128],
                                rhs,
                                start=(idx == 0),
                                stop=(idx == 17),
                                skip_group_check=True,
                            )
                        idx += 1
                for half in range(2):
                    oc_t = p_oc.tile([128, 4, 256], bf16, tag="oc",
                                     name="oc_t")
                    for j in range(2):
                        nc.scalar.copy(
                            oc_t[:, 2 * j : 2 * j + 2, :],
                            pss[half * 2 + j][:],
                        )
                    nc.sync.dma_start(
                        outc_d[bb, mt][:, grp * 8 + half * 4 :][:, :4, :],
                        oc_t[:],
                    )

            for ci, (c0, Lv) in enumerate(_chunks(VB, VCHUNK)):
                if ci == 1:
                    emit_fold()
                LX = Lv + 2
                x_c = p_x2.tile([128, CT, LX, 256], fp32r, tag="xc")
                for kt in range(CT):
                    nc.sync.dma_start(
                        x_c[:, kt], x_d[b, kt][:, c0 : c0 + LX, :]
                    )

                for t in range(CT):
                    u_t = p_u.tile([128, LX, 258], bf16, tag="ut")
                    nc.gpsimd.memset(u_t[:, :, 0], 0.0)
                    nc.gpsimd.memset(u_t[:, :, 257], 0.0)
                    for n in range(LX // 2):
                        ps = ps_conv2.tile([128, 2, 256], fp32, tag="psc2")
                        for kt in range(CT):
                            rhs = x_c[:, kt, 2 * n : 2 * n + 2, :]
                            nc.tensor.matmul(
                                ps[:],
                                wq[:, kt, (QKCT + t) * 128 :][:, :128],
                                rhs,
                                start=(kt == 0),
                                stop=(kt == CT - 1),
                            )
                        nc.scalar.copy(u_t[:, 2 * n : 2 * n + 2, 1:257], ps[:])
                    v3 = vband[:, t, c0 : c0 + Lv, 1:257]

                    def winv(tp, LL=Lv, src=u_t):
                        dy, dx = tp // 3, tp % 3
                        return src[:, dy : dy + LL, dx : dx + 256]

                    def scv(tp, tt=t):
                        return wdw[:, QKCT + tt, tp : tp + 1]

                    dyi, dxi = V_INIT // 3, V_INIT % 3
                    for n in range(Lv // 2):
                        psd = ps_dw2.tile([128, 2, 256], fp32, tag="psdw")
                        for j, tp in enumerate(V_PE_TAPS):
                            dy, dx = tp // 3, tp % 3
                            rhs = u_t[:, 2 * n + dy : 2 * n + dy + 2,
                                      dx : dx + 256]
                            nc.tensor.matmul(
                                psd[:],
                                diags[(QKCT + t, tp)][:],
                                rhs,
                                start=(j == 0),
                                stop=(j == len(V_PE_TAPS) - 1),
                            )
                        nc.vector.scalar_tensor_tensor(
                            v3[:, 2 * n : 2 * n + 2, :],
                            u_t[:, 2 * n + dyi : 2 * n + dyi + 2,
                                dxi : dxi + 256],
                            scv(V_INIT),
                            psd[:],
                            op0=Alu.mult, op1=Alu.add,
                        )
                    for tp in V_ACT_TAPS:
                        ascr = p_ascr2.tile([128, Lv, 256], bf16, tag="ascr2")
                        nc.scalar.activation(ascr[:], winv(tp), Act.Copy,
                                             scale=scv(tp))
                        nc.vector.tensor_tensor(v3, v3, ascr[:], op=Alu.add)
                    for tp in V_DVE_PAIRS:
                        gscr = p_gscr2.tile([128, Lv, 256], bf16, tag="gscr2")
                        nc.vector.tensor_scalar_mul(gscr[:], winv(tp), scv(tp))
                        nc.vector.tensor_tensor(v3, v3, gscr[:], op=Alu.add)
                    for tp in V_GP_TAPS:
                        gscr = p_gscr2.tile([128, Lv, 256], bf16, tag="gscr2")
                        nc.gpsimd.tensor_scalar_mul(gscr[:], winv(tp), scv(tp))
                        nc.vector.tensor_tensor(v3, v3, gscr[:], op=Alu.add)
                    # vband output: owned band rows [1, 33) only
                    lo = max(c0, 1)
                    hi = min(c0 + Lv, VB - 1)
                    if hi > lo:
                        nc.sync.dma_start(
                            vband_d[b, t][:, lo:hi, :],
                            vband[:, t, lo:hi, 1:257],
                        )
                    # halo masking at image edges (proj input only)
                    if c0 == 0:
                        nc.vector.tensor_scalar_mul(
                            vband[:, t, 0, 1:257], vband[:, t, 0, 1:257],
                            hmask[:, 0:1],
                        )
                    if c0 + Lv == VB:
                        nc.vector.tensor_scalar_mul(
                            vband[:, t, VB - 1, 1:257],
                            vband[:, t, VB - 1, 1:257],
                            hmask[:, 1:2],
                        )

                if ci >= 1:
                    for mt in range(CT):
                        proj_group(mt, ci - 1)


# ------------------------------------------------------------------- host side
_NC_CACHE = None


def _get_program():
    global _NC_CACHE
    if _NC_CACHE is None:
        _NC_CACHE = build_program()
    return _NC_CACHE


def kernel(x_in, w_qkv, w_dw, rescale, w_proj):
    x_in = np.asarray(x_in, dtype=np.float32)
    w_qkv = np.asarray(w_qkv, dtype=np.float32)
    w_dw = np.asarray(w_dw, dtype=np.float32)
    rescale = np.asarray(rescale, dtype=np.float32)
    w_proj = np.asarray(w_proj, dtype=np.float32)

    # x: NHWC -> NCHW, pad 2 halo rows top/bottom
    xT = np.transpose(x_in, (0, 3, 1, 2))                    # [B, C, H, W]
    xpad = np.zeros((B, C, H + 4, W), np.float32)
    xpad[:, :, 2 : H + 2, :] = xT

    # weights in device layouts
    wq_h = w_qkv[:, :, 0, 0]                                 # [768, 256]
    wq_l = np.ascontiguousarray(
        wq_h.T.reshape(CT, 128, 768).transpose(1, 0, 2)
    )                                                        # [128, CT, 768]
    wdw_l = np.ascontiguousarray(
        w_dw[:, 0].reshape(6, 128, 9).transpose(1, 0, 2)
    )                                                        # [128, 6, 9]
    wp_l = np.ascontiguousarray(
        w_proj.transpose(1, 2, 3, 0)                         # [i, 3, 3, o]
        .reshape(C, 9, C)
        .reshape(CT, 128, 9, C)
        .transpose(1, 0, 2, 3)
    )                                                        # [128, CT, 9, 256]
    ident = np.eye(128, dtype=np.float32)
    resc_l = np.empty((128, CT), np.float32)
    r = rescale.reshape(HEADS)
    for g in range(CT):
        resc_l[:, g] = np.repeat(r[4 * g : 4 * g + 4], 32)

    in_maps = []
    for i in range(N_CORES):
        band = np.ascontiguousarray(
            xpad[:, :, 32 * i : 32 * i + XB, :]
        ).reshape(B, CT, 128, XB, 256)
        hm = np.ones((128, 2), np.float32)
        if i == 0:
            hm[:, 0] = 0.0
        if i == N_CORES - 1:
            hm[:, 1] = 0.0
        in_maps.append(
            {
                "x": band,
                "wq": wq_l,
                "wdw": wdw_l,
                "wp": wp_l,
                "ident": ident,
                "resc": resc_l,
                "hmask": hm,
            }
        )

    nc = _get_program()
    res = bass_utils.run_bass_kernel_spmd(
        nc, in_maps, core_ids=list(range(N_CORES))
    )

    v_out = np.empty((B, C, H, W), np.float32)
    outc = np.empty((B, C, H, W), np.float32)
    for i in range(N_CORES):
        vb = np.asarray(res.results[i]["vband"]).astype(np.float32)
        oc = np.asarray(res.results[i]["outc"]).astype(np.float32)
        v_out[:, :, 32 * i : 32 * i + 32, :] = vb[:, :, :, 1:33, :].reshape(
            B, C, 32, 256
        )
        outc[:, :, 32 * i : 32 * i + 32, :] = oc.reshape(B, C, 32, 256)

    out_c = np.ascontiguousarray(np.transpose(outc, (0, 2, 3, 1)))
    return (out_c, v_out)
